# revision 10
# baseline (speedup 1.0000x reference)
"""Trainium2 Bass kernel for nn_Attention_12189117186326 (sparse causal attention).

Sharding: tensor-parallel over heads — 16 heads / 8 cores = 2 heads per core,
both batch elements on every core.  Per-core partial outputs (head-slice of the
output projection, head-sum of the attention matrix) are combined on the host.

Per-core math (heads h0=2c, h0+1), all matmuls in float32r (PE-rounded fp32):
  qT,kT  [64, n]   = Wq/Wk-slice^T @ x^T          (scores layout, q pre-scaled)
  vN     [n, 65]   = (x @ Wv-slice | ones)        (ones column -> softmax denom)
  sT     [128j, 512i] = kT-block^T . qT-chunk     (transposed scores, causal trapezoid only)
  p      = exp(sT + keymask_j) * ebT              (ebT = exp(pos_bias)^T, host-precomputed,
                                                   zeroed at j>i -> causal mask for free)
  oT|den [65, 512i]  += vN-block^T . p            (row 64 = softmax denominator)
  attnT  += p * (1/den)                           (1/den broadcast via PE outer product)
  outp   [n, 1024] += oT-block^T . Wout-slice
"""
import numpy as np

B, N, DIM, H, DH = 2, 2048, 1024, 16, 64
INNER = H * DH
N_CORES = 8
HPC = 2              # heads per core
P = 128              # partitions / j-block
CI = 512             # i-chunk width (one PSUM bank of fp32)
NEG = -30000.0       # key-mask additive constant (exp underflows to exactly 0)
KT = DIM // P        # k-tiles in the projection contractions

_nc_cache = {}


def _build():
    import concourse.tile as tile
    from concourse import bacc, mybir

    f32 = mybir.dt.float32
    f32r = mybir.dt.float32r
    NB = N // P          # j-blocks
    NCI = N // CI        # i-chunks

    nc = bacc.Bacc("TRN2", target_bir_lowering=False, debug=False, num_devices=N_CORES)

    xT_d = nc.dram_tensor("xT", [DIM, B, N], f32r, kind="ExternalInput").ap()
    wqk_d = nc.dram_tensor("wqk", [DIM, HPC, P], f32r, kind="ExternalInput").ap()
    wv_d = nc.dram_tensor("wv", [DIM, HPC * DH], f32r, kind="ExternalInput").ap()
    wout_d = nc.dram_tensor("wout", [HPC, DH, DIM], f32r, kind="ExternalInput").ap()
    eb_d = nc.dram_tensor("eb", [HPC, N, N], f32, kind="ExternalInput").ap()
    km_d = nc.dram_tensor("km", [B, NB, P], f32, kind="ExternalInput").ap()
    onesr_d = nc.dram_tensor("onesr", [1, P], f32r, kind="ExternalInput").ap()
    onesv_d = nc.dram_tensor("onesv", [P, HPC, NB, 1], f32r, kind="ExternalInput").ap()
    outp_d = nc.dram_tensor("outp", [B, N, DIM], f32, kind="ExternalOutput").ap()
    attnT_d = nc.dram_tensor("attnT", [B, N, N], f32r, kind="ExternalOutput").ap()

    from concourse.masks import make_identity

    with tile.TileContext(nc) as tc:
        with tc.tile_pool(name="w", bufs=1) as wpool:
            wqk_sb = wpool.tile([P, KT, HPC, P], f32r, tag="wqk")
            nc.sync.dma_start(out=wqk_sb, in_=wqk_d.rearrange("(k p) h m -> p k h m", p=P))
            wv_sb = wpool.tile([P, KT, HPC * DH], f32r, tag="wv")
            nc.sync.dma_start(out=wv_sb, in_=wv_d.rearrange("(k p) m -> p k m", p=P))
            wout_sb = wpool.tile([DH, HPC, DIM], f32r, tag="wout")
            nc.sync.dma_start(out=wout_sb, in_=wout_d.rearrange("h p d -> p h d"))
            km_sb = wpool.tile([P, B, NB], f32, tag="km")
            nc.sync.dma_start(out=km_sb, in_=km_d.rearrange("b j p -> p b j"))
            ident = wpool.tile([P, P], f32, tag="ident")
            make_identity(nc, ident)
            # ones row living on partition 64 — matmul lhsT/rhs bases must match
            # the denominator row of the attention PSUM (partition DH=64)
            ones_t = wpool.tile([DH + 1, P], f32r, tag="ones")
            nc.sync.dma_start(out=ones_t[DH:DH + 1, :], in_=onesr_d)

            for b in range(B):
                with tc.tile_pool(name="qkv", bufs=1) as qkvpool:
                    qT_sb = qkvpool.tile([DH, HPC, N], f32r, tag="qT")
                    kT_sb = qkvpool.tile([DH, HPC, N], f32r, tag="kT")
                    vN_sb = qkvpool.tile([P, HPC, NB, DH + 1], f32r, tag="vN")
                    nc.sync.dma_start(out=vN_sb[:, :, :, DH:DH + 1], in_=onesv_d)
                    oT_sb = qkvpool.tile([DH, HPC, N], f32r, tag="oT")

                    # ---- phase 1: q/k/v projections for this batch ----
                    with tc.tile_pool(name="xt", bufs=2) as xpool, \
                         tc.tile_pool(name="psq", bufs=2, space="PSUM") as psq, \
                         tc.tile_pool(name="psv", bufs=2, space="PSUM") as psv, \
                         tc.tile_pool(name="pst", bufs=2, space="PSUM") as pstp, \
                         tc.tile_pool(name="vtmp", bufs=2) as vtpool:
                        for cc in range(NCI):
                            cs = cc * CI
                            xt = xpool.tile([P, KT, CI], f32r, tag="xt")
                            nc.sync.dma_start(
                                out=xt,
                                in_=xT_d[:, b, cs:cs + CI].rearrange("(k p) n -> p k n", p=P))
                            for hl in range(HPC):
                                for qk in range(2):  # 0 -> q rows, 1 -> k rows
                                    ps = psq.tile([DH, CI], f32, tag="psqk")
                                    for k in range(KT):
                                        nc.tensor.matmul(
                                            out=ps[:],
                                            lhsT=wqk_sb[:, k, hl, qk * DH:(qk + 1) * DH],
                                            rhs=xt[:, k, :],
                                            start=(k == 0), stop=(k == KT - 1))
                                    dst = qT_sb if qk == 0 else kT_sb
                                    nc.scalar.copy(out=dst[:, hl, cs:cs + CI], in_=ps[:])
                            # vT chunk [128(2h*64), 512] then transpose to natural
                            psvt = psv.tile([P, CI], f32, tag="psvt")
                            for k in range(KT):
                                nc.tensor.matmul(out=psvt[:], lhsT=wv_sb[:, k, :],
                                                 rhs=xt[:, k, :],
                                                 start=(k == 0), stop=(k == KT - 1))
                            vt = vtpool.tile([P, CI], f32, tag="vt")
                            nc.vector.tensor_copy(out=vt[:], in_=psvt[:])
                            for t in range(CI // P):
                                ptr = pstp.tile([P, P], f32, tag="ptr")
                                nc.tensor.transpose(ptr[:], vt[:, t * P:(t + 1) * P], ident[:])
                                jb = cc * (CI // P) + t
                                for hl in range(HPC):
                                    nc.vector.tensor_copy(
                                        out=vN_sb[:, hl, jb, 0:DH],
                                        in_=ptr[:, hl * DH:(hl + 1) * DH])

                    # ---- phase 2: attention for this batch ----
                    with tc.tile_pool(name="eb", bufs=3) as ebpool, \
                         tc.tile_pool(name="pp", bufs=36) as ppool, \
                         tc.tile_pool(name="rc", bufs=4) as rcpool, \
                         tc.tile_pool(name="pssc", bufs=2, space="PSUM") as pssc, \
                         tc.tile_pool(name="psat", bufs=2, space="PSUM") as psat, \
                         tc.tile_pool(name="psrb", bufs=2, space="PSUM") as psrb:
                        for ci in range(NCI):
                            cs = ci * CI
                            nj = (ci + 1) * (CI // P)
                            acc = None
                            for hl in range(HPC):
                                pa = psat.tile([DH + 1, CI], f32, tag="pa")
                                ptiles = []
                                for jb in range(nj):
                                    ps = pssc.tile([P, CI], f32, tag="pssc")
                                    nc.tensor.matmul(
                                        out=ps[:],
                                        lhsT=kT_sb[:, hl, jb * P:(jb + 1) * P],
                                        rhs=qT_sb[:, hl, cs:cs + CI],
                                        start=True, stop=True)
                                    ebt = ebpool.tile([P, CI], f32, tag="eb")
                                    nc.sync.dma_start(
                                        out=ebt,
                                        in_=eb_d[hl, jb * P:(jb + 1) * P, cs:cs + CI])
                                    p = ppool.tile([P, CI], f32r, tag="p")
                                    nc.scalar.activation(
                                        out=p[:], in_=ps[:],
                                        func=mybir.ActivationFunctionType.Exp,
                                        bias=km_sb[:, b, jb:jb + 1], scale=1.0)
                                    nc.vector.tensor_mul(p[:], p[:], ebt[:])
                                    nc.tensor.matmul(
                                        out=pa[:], lhsT=vN_sb[:, hl, jb, :], rhs=p[:],
                                        start=(jb == 0), stop=(jb == nj - 1))
                                    ptiles.append(p)
                                # denominator -> reciprocal (stays on partition 64) ->
                                # broadcast to all 128 partitions via PE outer product
                                dn = rcpool.tile([DH + 1, CI], f32r, tag="dn")
                                with nc.allow_low_precision(reason="f32r is fp32-width"):
                                    nc.vector.reciprocal(dn[DH:DH + 1, :], pa[DH:DH + 1, :])
                                rcb = psrb.tile([P, CI], f32, tag="rcb")
                                nc.tensor.matmul(out=rcb[:], lhsT=ones_t[DH:DH + 1, :],
                                                 rhs=dn[DH:DH + 1, :], start=True, stop=True)
                                rcs = rcpool.tile([P, CI], f32, tag="rcs")
                                nc.scalar.copy(out=rcs[:], in_=rcb[:])
                                # normalized head output chunk
                                nc.vector.tensor_mul(
                                    oT_sb[:, hl, cs:cs + CI], pa[0:DH, :], rcs[0:DH, :])
                                # normalize p tiles; accumulate heads; store attnT
                                if hl == 0:
                                    acc = ptiles
                                    for jb in range(nj):
                                        nc.vector.tensor_mul(acc[jb][:], acc[jb][:], rcs[:])
                                else:
                                    for jb in range(nj):
                                        nc.vector.tensor_mul(ptiles[jb][:], ptiles[jb][:], rcs[:])
                                        nc.vector.tensor_add(acc[jb][:], acc[jb][:], ptiles[jb][:])
                                        nc.sync.dma_start(
                                            out=attnT_d[b, jb * P:(jb + 1) * P, cs:cs + CI],
                                            in_=acc[jb][:])

                    # ---- phase 3: output projection for this batch ----
                    with tc.tile_pool(name="pspr", bufs=4, space="PSUM") as pspr, \
                         tc.tile_pool(name="ob", bufs=4) as obpool:
                        for nb in range(N // P):
                            for half in range(DIM // CI):
                                ps = pspr.tile([P, CI], f32, tag="pspr")
                                for hl in range(HPC):
                                    nc.tensor.matmul(
                                        out=ps[:],
                                        lhsT=oT_sb[:, hl, nb * P:(nb + 1) * P],
                                        rhs=wout_sb[:, hl, half * CI:(half + 1) * CI],
                                        start=(hl == 0), stop=(hl == HPC - 1))
                                ob = obpool.tile([P, CI], f32, tag="ob")
                                nc.scalar.copy(out=ob[:], in_=ps[:])
                                nc.sync.dma_start(
                                    out=outp_d[b, nb * P:(nb + 1) * P, half * CI:(half + 1) * CI],
                                    in_=ob[:])
    nc.compile()
    return nc


def _get_nc():
    if "nc" not in _nc_cache:
        _nc_cache["nc"] = _build()
    return _nc_cache["nc"]


def _prep_inputs(x, mask, positions_bias, W_qkv, W_out):
    scale = np.float32(DH ** -0.5)
    x = np.asarray(x, np.float32)
    mask = np.asarray(mask)
    pb = np.asarray(positions_bias, np.float32)
    W_qkv = np.asarray(W_qkv, np.float32)
    W_out = np.asarray(W_out, np.float32)
    NB = N // P

    xT = np.ascontiguousarray(x.transpose(2, 0, 1))  # [DIM, B, N]
    Wq, Wk, Wv = W_qkv[:, :INNER], W_qkv[:, INNER:2 * INNER], W_qkv[:, 2 * INNER:]
    km = np.where(mask, np.float32(NEG), np.float32(0.0)).astype(np.float32)
    km = np.ascontiguousarray(km.reshape(B, NB, P))

    # ebT[h, j, i] = exp(pos_bias[h, i, j]), zeroed where j > i (causal)
    ebT = np.exp(pb[0]).transpose(0, 2, 1)  # [H, j, i]
    tri = np.tri(N, dtype=bool).T           # [j, i], True where j <= i
    ebT = np.where(tri[None, :, :], ebT, np.float32(0.0)).astype(np.float32)

    in_maps = []
    for c in range(N_CORES):
        h0 = HPC * c
        wqk = np.empty((DIM, HPC, P), np.float32)
        for hl in range(HPC):
            h = h0 + hl
            wqk[:, hl, :DH] = Wq[:, h * DH:(h + 1) * DH] * scale
            wqk[:, hl, DH:] = Wk[:, h * DH:(h + 1) * DH]
        wv = np.ascontiguousarray(
            np.concatenate([Wv[:, (h0 + hl) * DH:(h0 + hl + 1) * DH] for hl in range(HPC)], axis=1))
        wout = np.ascontiguousarray(
            np.stack([W_out[(h0 + hl) * DH:(h0 + hl + 1) * DH, :] for hl in range(HPC)]))
        eb = np.ascontiguousarray(ebT[h0:h0 + HPC])
        in_maps.append({"xT": xT, "wqk": wqk, "wv": wv, "wout": wout, "eb": eb, "km": km,
                        "onesr": np.ones((1, P), np.float32),
                        "onesv": np.ones((P, HPC, N // P, 1), np.float32)})
    return in_maps


def kernel(x, mask, positions_bias, W_qkv, W_out, b_out, _trace=False):
    from concourse.bass_utils import run_bass_kernel_spmd
    nc = _get_nc()
    in_maps = _prep_inputs(x, mask, positions_bias, W_qkv, W_out)
    res = run_bass_kernel_spmd(nc, in_maps, list(range(N_CORES)), trace=_trace)
    if _trace:
        _nc_cache["last_result"] = res
    outp = np.zeros((B, N, DIM), np.float32)
    attnT = np.zeros((B, N, N), np.float32)
    for r in res.results:
        outp += r["outp"]
        attnT += r["attnT"]
    outp += np.asarray(b_out, np.float32)
    attn_avg = np.ascontiguousarray(attnT.transpose(0, 2, 1)) / np.float32(H)
    return outp, attn_avg


# revision 11
# speedup vs baseline: 1.2014x; 1.2014x over previous
"""Trainium2 Bass kernel for nn_Attention_12189117186326 (sparse causal attention).

Sharding: tensor-parallel over heads — 16 heads / 8 cores = 2 heads per core,
both batch elements on every core.  Per-core partial outputs (head-slice of the
output projection, head-sum of the attention matrix) are combined on the host.

Per-core math (heads h0=2c, h0+1), matmuls in float32r (PE-rounded fp32):
  qT,kT  [64, n]   = Wq/Wk-slice^T @ x^T          (scores layout, q pre-scaled)
  vN     [n, 65]   = (x @ Wv-slice | ones)        (ones column -> softmax denom)
  sT     [128j, 512i] = kT-block^T . qT-chunk     (transposed scores, causal trapezoid only)
  sT    += I^T . biasT-tile                       (fp16 identity-matmul adds pos-bias
                                                   + causal -30000 on the PE, not DVE)
  p      = exp(sT + keymask_j)                    (keymask via ACT per-partition bias)
  oT|den [65, 512i]  += vN-block^T . p            (row 64 = softmax denominator)
  attnT  += p * (1/den)                           (1/den broadcast via PE outer product)
  outp   [n, 1024] += oT-block^T . Wout-slice

All large DMA streams use tile-major DRAM layouts (one contiguous burst per
tile); the host packs/unpacks.
"""
import numpy as np

B, N, DIM, H, DH = 2, 2048, 1024, 16, 64
INNER = H * DH
N_CORES = 8
HPC = 2              # heads per core
P = 128              # partitions / j-block
CI = 512             # i-chunk width (one PSUM bank of fp32)
NCI = N // CI        # 4 i-chunks
NB = N // P          # 16 j-blocks
NEG = -30000.0       # mask additive constant (exp underflows to exactly 0)
KT = DIM // P        # k-tiles in the projection contractions

# trapezoid tile enumeration: (ci, jb) for jb covering j <= i
TILES = [(ci, jb) for ci in range(NCI) for jb in range((ci + 1) * (CI // P))]
NT = len(TILES)      # 40
TILE_IDX = {t: n for n, t in enumerate(TILES)}

_nc_cache = {}


def _build():
    import concourse.tile as tile
    from concourse import bacc, mybir

    f32 = mybir.dt.float32
    f32r = mybir.dt.float32r
    f16 = mybir.dt.float16

    nc = bacc.Bacc("TRN2", target_bir_lowering=False, debug=False, num_devices=N_CORES)

    xT_d = nc.dram_tensor("xT", [B, NCI, P, KT, CI], f32r, kind="ExternalInput").ap()
    wqk_d = nc.dram_tensor("wqk", [DIM, HPC, P], f32r, kind="ExternalInput").ap()
    wv_d = nc.dram_tensor("wv", [DIM, HPC * DH], f32r, kind="ExternalInput").ap()
    wout_d = nc.dram_tensor("wout", [HPC, DH, DIM], f32r, kind="ExternalInput").ap()
    eb_d = nc.dram_tensor("eb", [HPC, NT, P, CI], f16, kind="ExternalInput").ap()
    km_d = nc.dram_tensor("km", [B, NB, P], f32, kind="ExternalInput").ap()
    idf_d = nc.dram_tensor("idf", [P, P], f16, kind="ExternalInput").ap()
    onesr_d = nc.dram_tensor("onesr", [1, P], f32r, kind="ExternalInput").ap()
    onesv_d = nc.dram_tensor("onesv", [P, HPC, NB, 1], f32r, kind="ExternalInput").ap()
    outp_d = nc.dram_tensor("outp", [B, NB, DIM // CI, P, CI], f32, kind="ExternalOutput").ap()
    attnT_d = nc.dram_tensor("attnT", [B, NT, P, CI], f32r, kind="ExternalOutput").ap()

    from concourse.masks import make_identity

    with tile.TileContext(nc) as tc:
        with tc.tile_pool(name="w", bufs=1) as wpool:
            wqk_sb = wpool.tile([P, KT, HPC, P], f32r, tag="wqk")
            nc.sync.dma_start(out=wqk_sb, in_=wqk_d.rearrange("(k p) h m -> p k h m", p=P))
            wv_sb = wpool.tile([P, KT, HPC * DH], f32r, tag="wv")
            nc.sync.dma_start(out=wv_sb, in_=wv_d.rearrange("(k p) m -> p k m", p=P))
            wout_sb = wpool.tile([DH, HPC, DIM], f32r, tag="wout")
            nc.sync.dma_start(out=wout_sb, in_=wout_d.rearrange("h p d -> p h d"))
            km_sb = wpool.tile([P, B, NB], f32, tag="km")
            nc.sync.dma_start(out=km_sb, in_=km_d.rearrange("b j p -> p b j"))
            ident = wpool.tile([P, P], f32, tag="ident")
            make_identity(nc, ident)
            idf_sb = wpool.tile([P, P], f16, tag="idf")
            nc.sync.dma_start(out=idf_sb, in_=idf_d)
            # ones row living on partition 64 — matmul lhsT/rhs bases must match
            # the denominator row of the attention PSUM (partition DH=64)
            ones_t = wpool.tile([DH + 1, P], f32r, tag="ones")
            nc.sync.dma_start(out=ones_t[DH:DH + 1, :], in_=onesr_d)

            for b in range(B):
                with tc.tile_pool(name="qkv", bufs=1) as qkvpool:
                    qT_sb = qkvpool.tile([DH, HPC, N], f32r, tag="qT")
                    kT_sb = qkvpool.tile([DH, HPC, N], f32r, tag="kT")
                    vN_sb = qkvpool.tile([P, HPC, NB, DH + 1], f32r, tag="vN")
                    nc.sync.dma_start(out=vN_sb[:, :, :, DH:DH + 1], in_=onesv_d)
                    oT_sb = qkvpool.tile([DH, HPC, N], f32r, tag="oT")

                    # ---- phase 1: q/k/v projections for this batch ----
                    with tc.tile_pool(name="xt", bufs=2) as xpool, \
                         tc.tile_pool(name="psq", bufs=2, space="PSUM") as psq, \
                         tc.tile_pool(name="psv", bufs=2, space="PSUM") as psv, \
                         tc.tile_pool(name="pst", bufs=2, space="PSUM") as pstp, \
                         tc.tile_pool(name="vtmp", bufs=2) as vtpool:
                        for cc in range(NCI):
                            cs = cc * CI
                            xt = xpool.tile([P, KT, CI], f32r, tag="xt")
                            nc.sync.dma_start(out=xt, in_=xT_d[b, cc])
                            for hl in range(HPC):
                                for qk in range(2):  # 0 -> q rows, 1 -> k rows
                                    ps = psq.tile([DH, CI], f32, tag="psqk")
                                    for k in range(KT):
                                        nc.tensor.matmul(
                                            out=ps[:],
                                            lhsT=wqk_sb[:, k, hl, qk * DH:(qk + 1) * DH],
                                            rhs=xt[:, k, :],
                                            start=(k == 0), stop=(k == KT - 1))
                                    dst = qT_sb if qk == 0 else kT_sb
                                    nc.scalar.copy(out=dst[:, hl, cs:cs + CI], in_=ps[:])
                            # vT chunk [128(2h*64), 512] then transpose to natural
                            psvt = psv.tile([P, CI], f32, tag="psvt")
                            for k in range(KT):
                                nc.tensor.matmul(out=psvt[:], lhsT=wv_sb[:, k, :],
                                                 rhs=xt[:, k, :],
                                                 start=(k == 0), stop=(k == KT - 1))
                            vt = vtpool.tile([P, CI], f32, tag="vt")
                            nc.vector.tensor_copy(out=vt[:], in_=psvt[:])
                            for t in range(CI // P):
                                ptr = pstp.tile([P, P], f32, tag="ptr")
                                nc.tensor.transpose(ptr[:], vt[:, t * P:(t + 1) * P], ident[:])
                                jb = cc * (CI // P) + t
                                for hl in range(HPC):
                                    nc.vector.tensor_copy(
                                        out=vN_sb[:, hl, jb, 0:DH],
                                        in_=ptr[:, hl * DH:(hl + 1) * DH])

                    # ---- phase 2: attention for this batch ----
                    with tc.tile_pool(name="eb", bufs=6) as ebpool, \
                         tc.tile_pool(name="pp", bufs=36) as ppool, \
                         tc.tile_pool(name="rc", bufs=4) as rcpool, \
                         tc.tile_pool(name="pssc", bufs=3, space="PSUM") as pssc, \
                         tc.tile_pool(name="psat", bufs=2, space="PSUM") as psat, \
                         tc.tile_pool(name="psrb", bufs=2, space="PSUM") as psrb:
                        for ci in range(NCI):
                            cs = ci * CI
                            nj = (ci + 1) * (CI // P)
                            acc = None
                            for hl in range(HPC):
                                pa = psat.tile([DH + 1, CI], f32, tag="pa")
                                ptiles = []
                                for jb in range(nj):
                                    ps = pssc.tile([P, CI], f32, tag="pssc")
                                    nc.tensor.matmul(
                                        out=ps[:],
                                        lhsT=kT_sb[:, hl, jb * P:(jb + 1) * P],
                                        rhs=qT_sb[:, hl, cs:cs + CI],
                                        start=True, stop=False)
                                    ebt = ebpool.tile([P, CI], f16, tag="eb")
                                    nc.sync.dma_start(out=ebt, in_=eb_d[hl, TILE_IDX[(ci, jb)]])
                                    nc.tensor.matmul(
                                        out=ps[:], lhsT=idf_sb[:], rhs=ebt[:],
                                        start=False, stop=True)
                                    p = ppool.tile([P, CI], f32r, tag="p")
                                    nc.scalar.activation(
                                        out=p[:], in_=ps[:],
                                        func=mybir.ActivationFunctionType.Exp,
                                        bias=km_sb[:, b, jb:jb + 1], scale=1.0)
                                    nc.tensor.matmul(
                                        out=pa[:], lhsT=vN_sb[:, hl, jb, :], rhs=p[:],
                                        start=(jb == 0), stop=(jb == nj - 1))
                                    ptiles.append(p)
                                # denominator -> reciprocal (stays on partition 64) ->
                                # broadcast to all 128 partitions via PE outer product
                                dn = rcpool.tile([DH + 1, CI], f32r, tag="dn")
                                with nc.allow_low_precision(reason="f32r is fp32-width"):
                                    nc.vector.reciprocal(dn[DH:DH + 1, :], pa[DH:DH + 1, :])
                                rcb = psrb.tile([P, CI], f32, tag="rcb")
                                nc.tensor.matmul(out=rcb[:], lhsT=ones_t[DH:DH + 1, :],
                                                 rhs=dn[DH:DH + 1, :], start=True, stop=True)
                                rcs = rcpool.tile([P, CI], f32, tag="rcs")
                                nc.scalar.copy(out=rcs[:], in_=rcb[:])
                                # normalized head output chunk
                                nc.vector.tensor_mul(
                                    oT_sb[:, hl, cs:cs + CI], pa[0:DH, :], rcs[0:DH, :])
                                # normalize p tiles; accumulate heads; store attnT
                                if hl == 0:
                                    acc = ptiles
                                    for jb in range(nj):
                                        nc.vector.tensor_mul(acc[jb][:], acc[jb][:], rcs[:])
                                else:
                                    for jb in range(nj):
                                        nc.vector.tensor_mul(ptiles[jb][:], ptiles[jb][:], rcs[:])
                                        nc.vector.tensor_add(acc[jb][:], acc[jb][:], ptiles[jb][:])
                                        nc.sync.dma_start(
                                            out=attnT_d[b, TILE_IDX[(ci, jb)]],
                                            in_=acc[jb][:])

                    # ---- phase 3: output projection for this batch ----
                    with tc.tile_pool(name="pspr", bufs=4, space="PSUM") as pspr, \
                         tc.tile_pool(name="ob", bufs=4) as obpool:
                        for nb in range(NB):
                            for half in range(DIM // CI):
                                ps = pspr.tile([P, CI], f32, tag="pspr")
                                for hl in range(HPC):
                                    nc.tensor.matmul(
                                        out=ps[:],
                                        lhsT=oT_sb[:, hl, nb * P:(nb + 1) * P],
                                        rhs=wout_sb[:, hl, half * CI:(half + 1) * CI],
                                        start=(hl == 0), stop=(hl == HPC - 1))
                                ob = obpool.tile([P, CI], f32, tag="ob")
                                nc.scalar.copy(out=ob[:], in_=ps[:])
                                nc.sync.dma_start(out=outp_d[b, nb, half], in_=ob[:])
    nc.compile()
    return nc


def _get_nc():
    if "nc" not in _nc_cache:
        _nc_cache["nc"] = _build()
    return _nc_cache["nc"]


def _prep_inputs(x, mask, positions_bias, W_qkv, W_out):
    scale = np.float32(DH ** -0.5)
    x = np.asarray(x, np.float32)
    mask = np.asarray(mask)
    pb = np.asarray(positions_bias, np.float32)
    W_qkv = np.asarray(W_qkv, np.float32)
    W_out = np.asarray(W_out, np.float32)

    # x tiled: xT_tiles[b, cc, p, k, n] = x[b, cc*CI+n, k*P+p]
    xT = np.ascontiguousarray(
        x.reshape(B, NCI, CI, KT, P).transpose(0, 1, 4, 3, 2))
    Wq, Wk, Wv = W_qkv[:, :INNER], W_qkv[:, INNER:2 * INNER], W_qkv[:, 2 * INNER:]
    km = np.where(mask, np.float32(NEG), np.float32(0.0)).astype(np.float32)
    km = np.ascontiguousarray(km.reshape(B, NB, P))

    # biasT[h, j, i] = pos_bias[h, i, j], NEG where j > i (causal); fp16, tile-major
    bT = pb[0].transpose(0, 2, 1)  # [H, j, i]
    tri = np.tri(N, dtype=bool).T  # [j, i], True where j <= i
    bT = np.where(tri[None, :, :], bT, np.float32(NEG)).astype(np.float16)
    eb_all = np.empty((H, NT, P, CI), np.float16)
    for t, (ci, jb) in enumerate(TILES):
        eb_all[:, t] = bT[:, jb * P:(jb + 1) * P, ci * CI:(ci + 1) * CI]

    in_maps = []
    for c in range(N_CORES):
        h0 = HPC * c
        wqk = np.empty((DIM, HPC, P), np.float32)
        for hl in range(HPC):
            h = h0 + hl
            wqk[:, hl, :DH] = Wq[:, h * DH:(h + 1) * DH] * scale
            wqk[:, hl, DH:] = Wk[:, h * DH:(h + 1) * DH]
        wv = np.ascontiguousarray(
            np.concatenate([Wv[:, (h0 + hl) * DH:(h0 + hl + 1) * DH] for hl in range(HPC)], axis=1))
        wout = np.ascontiguousarray(
            np.stack([W_out[(h0 + hl) * DH:(h0 + hl + 1) * DH, :] for hl in range(HPC)]))
        in_maps.append({"xT": xT, "wqk": wqk, "wv": wv, "wout": wout,
                        "eb": np.ascontiguousarray(eb_all[h0:h0 + HPC]), "km": km,
                        "idf": np.eye(P, dtype=np.float16),
                        "onesr": np.ones((1, P), np.float32),
                        "onesv": np.ones((P, HPC, NB, 1), np.float32)})
    return in_maps


def kernel(x, mask, positions_bias, W_qkv, W_out, b_out, _trace=False):
    from concourse.bass_utils import run_bass_kernel_spmd
    nc = _get_nc()
    in_maps = _prep_inputs(x, mask, positions_bias, W_qkv, W_out)
    res = run_bass_kernel_spmd(nc, in_maps, list(range(N_CORES)), trace=_trace)
    if _trace:
        _nc_cache["last_result"] = res
    outp_t = np.zeros((B, NB, DIM // CI, P, CI), np.float32)
    attnT_t = np.zeros((B, NT, P, CI), np.float32)
    for r in res.results:
        outp_t += r["outp"]
        attnT_t += r["attnT"]
    outp = outp_t.transpose(0, 1, 3, 2, 4).reshape(B, N, DIM)
    outp += np.asarray(b_out, np.float32)
    attn_avg = np.zeros((B, N, N), np.float32)
    for t, (ci, jb) in enumerate(TILES):
        attn_avg[:, ci * CI:(ci + 1) * CI, jb * P:(jb + 1) * P] = \
            attnT_t[:, t].transpose(0, 2, 1)
    attn_avg /= np.float32(H)
    return np.ascontiguousarray(outp), attn_avg
